# revision 1
# baseline (speedup 1.0000x reference)
"""NSMCell message-passing kernel for 8 Trainium2 NeuronCores.

Contract: kernel(**inputs) takes the FULL unsharded inputs (numpy/jax arrays)
and returns the FULL (N,) float32 output, matching reference.reference().

Math restructuring (exact, up to float assoc.):
  edge path:  msg @ w_rel = segment_sum(dist[src] * (elu((i_b*a_e)@W_edge) @ w_rel), dst)
              and (i_b (*) a_e) @ W_edge = a_e @ (diag(i_b) @ W_edge) = a_e @ U_b
    -> per-edge scalar t_e = w_rel . elu(a_e @ U_b),  b = edge_batch_indices[e]
  node path:  s_n = w_node . elu(sum_p attr[n,p] @ V_{b,p}),
              V_{b,p} = sim[b,p] * diag(i_b) @ W_props[p],  b = node_indices[n]
  host epilogue (O(N+E) scalar work): scatter-add t into nodes by dst,
  two segment softmaxes over graphs, final mix by relation_similarity.

Device work = all the heavy lifting (the E*H*H and N*P*H*H matmuls + elu over
the (E,H)/(N,H) intermediates), sharded: edges sorted by graph id and dealt
8 ways; nodes (already sorted by graph) dealt 8 ways. Single SPMD program.

elu on device: ACT computes e = exp(z) (f32 internal, from PSUM); a custom
DVE op computes elu(z) = relu(z) + min(e, 1) - 1 in one fused pass.
"""

import os
import sys
import types

import numpy as np

# ---------------------------------------------------------------------------
# problem constants (hardcoded per contract)
N, P, H, E, B = 100000, 4, 128, 1000000, 64
NCORES = 8
TZ = 512          # matmul moving-dim tile (one PSUM bank of f32)
ZCOLS = 1024      # z PSUM tile free size (2 banks) = 2 matmuls
TACC_COLS = 512   # t-accumulator PSUM tile (1 bank)

_DT = os.environ.get("KERNEL_DTYPE", "fp16")  # fp16 | fp32


# ---------------------------------------------------------------------------
def _install_ntff_hook():
    """Allow BASS_TRACE=1 profiling under axon (test.py); harmless otherwise."""
    try:
        from antenv.axon_hooks import get_axon_ntff_profile_hook  # noqa: F401
        return
    except ImportError:
        pass
    try:
        from trn_agent_boot.trn_boot import _ntff_profile_via_ctypes
        hook = _ntff_profile_via_ctypes("/opt/axon/libaxon_pjrt.so")
    except Exception:
        hook = None
    mod = types.ModuleType("antenv.axon_hooks")
    _state = {"hook": hook}
    mod.get_axon_ntff_profile_hook = lambda: _state["hook"]
    mod.set_axon_ntff_profile_hook = lambda h: _state.__setitem__("hook", h)
    sys.modules["antenv.axon_hooks"] = mod
    try:
        import antenv
        antenv.axon_hooks = mod
    except ImportError:
        pass


def _make_elu_op():
    """Register custom DVE op: out = s0 * (relu(in0) + min(in1, 1) - 1)
    (= s0 * elu(in0) when in1 == exp(in0); s0 is a per-partition weight AP).
    Runtime registration: append to dve_ops.OPS."""
    from concourse import dve_ops
    from concourse.dve_spec import (Spec, Src0, Src1, C0, One, relu, minn,
                                    lower)
    from concourse.dve_uop import DveOpSpec

    name = "WELU_FROM_EXP_ANT"
    for op in dve_ops.OPS:
        if op.name == name:
            return op
    spec = Spec(
        body=(relu(Src0) + minn(Src1, One) - One) * C0,
        reference=lambda in0, in1, s0, s1, imm2: (
            (np.maximum(np.nan_to_num(in0, nan=0.0), 0)
             + np.minimum(in1, np.float32(1.0))
             - np.float32(1.0)) * s0
        ).astype(np.float32),
    )
    row = dve_ops._CUSTOM_DVE_ROW_BASE + len(dve_ops.OPS)
    assert row < 0x20
    shas = {}
    for ver in ("v3", "v4"):
        shas[ver] = DveOpSpec(
            name=name, opcode=row, uops=lower(spec, ver=ver), rd1_en=True
        ).sha(ver)
    op = dve_ops.DveOp(name, spec, subdim=False, uops_sha=shas)
    dve_ops.OPS.append(op)
    dve_ops.CUSTOM_DVE_SPECS[name] = spec
    dve_ops._SUB_OPCODE_FOR_NAME[name] = row
    return op


# ---------------------------------------------------------------------------
def _build_program(me: int, mn: int, dt_lo, np_lo):
    """Build the SPMD bass program. Core c owns graphs [8c, 8c+8) whole.
    me = per-graph padded edge count (multiple of TZ); mn = per-graph padded
    node count."""
    import concourse.tile as tile
    from concourse import bacc
    import concourse.mybir as mybir

    f32 = mybir.dt.float32
    Exp = mybir.ActivationFunctionType.Exp
    elu_op = _make_elu_op()
    GB = B // NCORES                       # graphs per core

    # z-tile spans within one edge group (1024s then a 512 remainder)
    ztiles = []
    off = 0
    while off < me:
        w = min(ZCOLS, me - off)
        ztiles.append((off, w))
        off += w
    # ea sub-DMA pieces (4096-edge chunks, aligned with z tiles)
    pieces = []
    off = 0
    while off < me:
        w = min(4096, me - off)
        pieces.append((off, w))
        off += w
    # node z-tiles and matvec chunk starts
    ntiles = []
    off = 0
    while off < mn:
        w = min(TZ, mn - off)
        ntiles.append((off, w))
        off += w
    nchunks = []
    for t0, tw in ntiles:
        c0 = 0
        while c0 < tw:
            nchunks.append((t0 + c0, min(H, tw - c0)))
            c0 += H

    ncol_t = GB * me // H
    ncol_s = GB * len(nchunks)
    assert ncol_s <= TACC_COLS

    nc = bacc.Bacc("TRN2", target_bir_lowering=False, debug=False,
                   num_devices=NCORES)

    ea_in = nc.dram_tensor("ea_t", [GB, H, me], dt_lo, kind="ExternalInput")
    na_in = nc.dram_tensor("na_t", [GB, H, P, mn], dt_lo,
                           kind="ExternalInput")
    u_in = nc.dram_tensor("u_tab", [H, GB, H], dt_lo, kind="ExternalInput")
    v_in = nc.dram_tensor("v_tab", [H, GB * P, H], dt_lo,
                          kind="ExternalInput")
    wr_in = nc.dram_tensor("w_rel_bc", [H, 1], dt_lo, kind="ExternalInput")
    wn_in = nc.dram_tensor("w_node_bc", [H, 1], dt_lo, kind="ExternalInput")
    # t_out[p, col]: col = g*(me//H) + cc; edge pos in group = cc*H + p
    t_out = nc.dram_tensor("t_out", [H, ncol_t], f32, kind="ExternalOutput")
    # s_out[p, g*len(nchunks) + i]: node pos = nchunks[i][0] + p of graph g
    s_out = nc.dram_tensor("s_out", [H, ncol_s], f32, kind="ExternalOutput")

    with tile.TileContext(nc) as tc:
        with (
            tc.tile_pool(name="consts", bufs=1) as cpool,
            tc.tile_pool(name="ework", bufs=3) as epool,
            tc.tile_pool(name="nwork", bufs=3) as npool,
            tc.tile_pool(name="outs", bufs=3) as opool,
            tc.tile_pool(name="zpsum", bufs=3, space="PSUM") as zpool,
            tc.tile_pool(name="tpsum", bufs=2, space="PSUM") as tpool,
        ):
            # DMA queue is FIFO: only the small tables the first matmuls
            # need go ahead of the first edge loads; the big node-attr and
            # V-table loads are emitted per-graph behind them (each is
            # consumed one phase later, so it always arrives in time).
            u_sb = cpool.tile([H, GB, H], dt_lo)
            nc.sync.dma_start(u_sb[:], u_in.ap())
            wr_sb = cpool.tile([H, 1], dt_lo)
            nc.sync.dma_start(wr_sb[:], wr_in.ap())
            wn_sb = cpool.tile([H, 1], dt_lo)
            nc.sync.dma_start(wn_sb[:], wn_in.ap())
            v_sb = cpool.tile([H, GB * P, H], dt_lo)
            na_sb = cpool.tile([H, GB, P, mn], dt_lo)

            tcol = 0
            scol = 0
            tacc = tpool.tile([H, TACC_COLS], f32, tag="tacc")
            sacc = tpool.tile([H, TACC_COLS], f32, tag="tacc")
            for g in range(GB):
                # ---- edge group g ----
                # group 0's first piece split small so the compute chain
                # starts as soon as possible after NEFF launch
                gp = ([(0, 1024), (1024, 3072)] + pieces[1:]) if g == 0 \
                    else pieces
                ea_parts = {}
                for p0, pw in gp:
                    pt = epool.tile([H, 4096], dt_lo, tag="ea")
                    nc.sync.dma_start(pt[:, :pw],
                                      ea_in.ap()[g][:, p0:p0 + pw])
                    ea_parts[p0] = pt
                nc.sync.dma_start(na_sb[:, g], na_in.ap()[g])
                if g == 0:
                    nc.sync.dma_start(v_sb[:], v_in.ap())
                def emit_node_tile(t0, tw):
                    nonlocal scol
                    zn = zpool.tile([H, ZCOLS], f32, tag="z")
                    for p in range(P):
                        nc.tensor.matmul(
                            zn[:, :tw],
                            v_sb[:, g * P + p, :],
                            na_sb[:, g, p, t0:t0 + tw],
                            start=(p == 0), stop=(p == P - 1),
                        )
                    en = npool.tile([H, TZ], dt_lo, tag="en")
                    nc.scalar.activation(en[:, :tw], zn[:, :tw], Exp)
                    psn = npool.tile([H, TZ], dt_lo, tag="psn")
                    nc.vector._custom_dve(elu_op, out=psn[:, :tw],
                                          in0=zn[:, :tw], in1=en[:, :tw],
                                          s0=1.0)
                    for c0 in range(0, tw, H):
                        w = min(H, tw - c0)
                        nc.tensor.matmul(
                            sacc[:w, scol:scol + 1],
                            psn[:, c0:c0 + w],
                            wn_sb[:],
                            start=True, stop=True,
                        )
                        scol += 1

                # last group: node tiles first, so the kernel tail ends on
                # overlappable edge drains instead of the serial node chain
                if g == GB - 1:
                    for t0, tw in ntiles:
                        emit_node_tile(t0, tw)
                for zi, (z0, zw) in enumerate(ztiles):
                    z = zpool.tile([H, ZCOLS], f32, tag="z")
                    p0 = max(q0_ for q0_, qw_ in gp if q0_ <= z0)
                    pt = ea_parts[p0]
                    po = z0 - p0
                    for q0 in range(0, zw, TZ):
                        nc.tensor.matmul(
                            z[:, q0:q0 + TZ],
                            u_sb[:, g, :],
                            pt[:, po + q0:po + q0 + TZ],
                            start=True, stop=True,
                        )
                    e_sb = epool.tile([H, ZCOLS], dt_lo, tag="e")
                    nc.scalar.activation(e_sb[:, :zw], z[:, :zw], Exp)
                    psi = epool.tile([H, ZCOLS], dt_lo, tag="psi")
                    nc.vector._custom_dve(elu_op, out=psi[:, :zw],
                                          in0=z[:, :zw], in1=e_sb[:, :zw],
                                          s0=1.0)
                    for c0 in range(0, zw, H):
                        col = tcol % TACC_COLS
                        nc.tensor.matmul(
                            tacc[:, col:col + 1],
                            psi[:, c0:c0 + H],
                            wr_sb[:],
                            start=True, stop=True,
                        )
                        tcol += 1
                        if tcol % TACC_COLS == 0 or tcol == ncol_t:
                            rem = tcol % TACC_COLS or TACC_COLS
                            t_sb = opool.tile([H, TACC_COLS], f32,
                                              tag="tsb")
                            nc.scalar.copy(t_sb[:, :rem], tacc[:, :rem])
                            nc.sync.dma_start(
                                t_out.ap()[:, tcol - rem:tcol],
                                t_sb[:, :rem])
                            if tcol < ncol_t:
                                tacc = tpool.tile([H, TACC_COLS], f32,
                                                  tag="tacc")
                # node tiles after the group's edge tiles (interleaving them
                # into the edge stream measured much worse)
                if g < GB - 1:
                    for t0, tw in ntiles:
                        emit_node_tile(t0, tw)
            s_sb = opool.tile([H, TACC_COLS], f32, tag="tsb")
            nc.scalar.copy(s_sb[:, :ncol_s], sacc[:, :ncol_s])
            nc.sync.dma_start(s_out.ap()[:], s_sb[:, :ncol_s])

    nc.compile()
    return nc


# ---------------------------------------------------------------------------
def kernel(node_attrs, edge_attrs, instruction_batch, distribution,
           node_prop_similarities, relation_similarity,
           W_props, W_edge, w_node_score, w_rel_score,
           edge_indices, node_indices, edge_batch_indices):
    _install_ntff_hook()
    from concourse import bass_utils
    if os.environ.get("KERNEL_LDW_OPT", "0") == "1":
        _enable_ldw_opt()

    np_lo = np.float16 if _DT == "fp16" else np.float32

    na = np.asarray(node_attrs, np.float32)
    ea = np.asarray(edge_attrs, np.float32)
    ib = np.asarray(instruction_batch, np.float32)
    dist = np.asarray(distribution, np.float32)
    nps = np.asarray(node_prop_similarities, np.float32)
    rs = np.asarray(relation_similarity, np.float32)
    Wp = np.asarray(W_props, np.float32)
    We = np.asarray(W_edge, np.float32)
    wn = np.asarray(w_node_score, np.float32)
    wr = np.asarray(w_rel_score, np.float32)
    ei = np.asarray(edge_indices).astype(np.int64)
    ni = np.asarray(node_indices).astype(np.int64)
    ebi = np.asarray(edge_batch_indices).astype(np.int64)
    src, dst = ei[0], ei[1]
    GB = B // NCORES

    # ---- transformed weight tables (host, exact f32 then cast) ----
    U = ib[:, :, None] * We[None, :, :]                        # (B,H,H)
    V = (nps[:, :, None, None] * ib[:, None, :, None] *
         Wp[None, :, :, :])                                    # (B,P,H,H)
    U_t = np.ascontiguousarray(U.transpose(1, 0, 2)).astype(np_lo)  # (H,B,H)
    V_t = np.ascontiguousarray(
        V.reshape(B * P, H, H).transpose(1, 0, 2)).astype(np_lo)    # (H,BP,H)
    wr_bc = wr.reshape(H, 1).astype(np_lo)
    wn_bc = wn.reshape(H, 1).astype(np_lo)

    # ---- sharding: core c owns graphs [8c, 8c+8) whole ----
    order = np.argsort(ebi, kind="stable")
    ecounts = np.bincount(ebi, minlength=B)
    estarts = np.concatenate([[0], np.cumsum(ecounts)[:-1]])
    me = ((int(ecounts.max()) + TZ - 1) // TZ) * TZ            # per graph
    ea_lo = ea.astype(np_lo)
    ebuf = np.zeros((B, me, H), np_lo)
    for b in range(B):
        s, c = estarts[b], ecounts[b]
        ebuf[b, :c] = ea_lo[order[s:s + c]]
    # (B, me, H) -> (cores, GB, H, me)
    ea_t = np.ascontiguousarray(
        ebuf.reshape(NCORES, GB, me, H).transpose(0, 1, 3, 2))
    del ebuf

    ncounts = np.bincount(ni, minlength=B)
    nstarts = np.concatenate([[0], np.cumsum(ncounts)[:-1]])
    mn = (int(ncounts.max()) + 1) // 2 * 2
    na_lo = na.astype(np_lo)
    nbuf = np.zeros((B, mn, P, H), np_lo)
    for b in range(B):
        s, c = nstarts[b], ncounts[b]
        nbuf[b, :c] = na_lo[s:s + c]
    # (B, mn, P, H) -> (cores, GB, H, P, mn)
    na_t = np.ascontiguousarray(
        nbuf.reshape(NCORES, GB, mn, P, H).transpose(0, 1, 4, 3, 2))
    del nbuf

    import concourse.mybir as mybir
    dt_lo = mybir.dt.float16 if np_lo is np.float16 else mybir.dt.float32

    nc = _build_program(me, mn, dt_lo, np_lo)

    in_maps = []
    for c in range(NCORES):
        in_maps.append({
            "ea_t": ea_t[c],
            "na_t": na_t[c],
            "u_tab": np.ascontiguousarray(U_t[:, c * GB:(c + 1) * GB, :]),
            "v_tab": np.ascontiguousarray(
                V_t[:, c * GB * P:(c + 1) * GB * P, :]),
            "w_rel_bc": wr_bc,
            "w_node_bc": wn_bc,
        })

    res = bass_utils.run_bass_kernel_spmd(
        nc, in_maps, core_ids=list(range(NCORES)),
        trace=bool(os.environ.get("BASS_TRACE")),
        tmpdir=os.environ.get("KERNEL_TRACE_DIR") or None,
    )
    kernel.last_results = res  # for test.py profiling introspection

    # ---- host epilogue ----
    # t: t_dev[c][p, col]: g = col // cpg, cc = col % cpg (cpg = me//H);
    # graph = c*GB + g; edge pos in group = cc*H + p.
    cpg = me // H
    ncol_t = GB * cpg
    t_dev = np.stack([np.asarray(res.results[c]["t_out"])
                      for c in range(NCORES)])       # (8, H, ncol_t)
    col = np.arange(ncol_t)
    q = (col % cpg)[None, :] * H + np.arange(H)[:, None]   # (H, ncol_t)
    graph = (np.arange(NCORES)[:, None, None] * GB
             + (col // cpg)[None, None, :])          # (8, 1, ncol_t)
    graph = np.broadcast_to(graph, (NCORES, H, ncol_t))
    qg = np.broadcast_to(q[None], graph.shape)
    valid = qg < ecounts[graph]
    t_full = np.zeros(E, np.float64)
    t_full[order[estarts[graph[valid]] + qg[valid]]] = t_dev[valid]

    # s: s_dev[c][p, g*nch + i]: node pos = chunk_start[i] + p of graph
    ntiles = []
    off = 0
    while off < mn:
        w = min(TZ, mn - off)
        ntiles.append((off, w))
        off += w
    chunk_starts = []
    for t0, tw in ntiles:
        c0 = 0
        while c0 < tw:
            chunk_starts.append(t0 + c0)
            c0 += H
    nch = len(chunk_starts)
    ncol_s = GB * nch
    chunk_starts = np.asarray(chunk_starts)
    s_dev = np.stack([np.asarray(res.results[c]["s_out"])
                      for c in range(NCORES)])       # (8, H, ncol_s)
    scol = np.arange(ncol_s)
    spos = chunk_starts[scol % nch][None, :] + np.arange(H)[:, None]
    sgraph = (np.arange(NCORES)[:, None, None] * GB
              + (scol // nch)[None, None, :])
    sgraph = np.broadcast_to(sgraph, (NCORES, H, ncol_s))
    sposg = np.broadcast_to(spos[None], sgraph.shape)
    svalid = sposg < np.minimum(ncounts[sgraph], mn)
    s_full = np.zeros(N, np.float64)
    s_full[nstarts[sgraph[svalid]] + sposg[svalid]] = s_dev[svalid]

    # scatter-add edge scalars into nodes, then segment softmaxes
    acc = np.bincount(dst, weights=dist[src].astype(np.float64) * t_full,
                      minlength=N)

    def seg_softmax(x):
        m = np.full(B, -np.inf)
        np.maximum.at(m, ni, x)
        e = np.exp(x - m[ni])
        ssum = np.zeros(B, np.float64)
        np.add.at(ssum, ni, e)
        return e / ssum[ni]

    next_rel = seg_softmax(acc)
    next_states = seg_softmax(s_full)
    rsn = rs[ni].astype(np.float64)
    out = rsn * next_rel + (1.0 - rsn) * next_states
    return out.astype(np.float32)

